# revision 22
# baseline (speedup 1.0000x reference)
"""Trainium2 Bass kernel for GaussianProcessEmbeddingHead.

The reference computes:
    mu     = x @ W_mu.T + b_mu                      (B,N,E)
    sigma  = exp(0.5*(x @ W_logvar.T + b_logvar))   (B,N,E)
    K      = RBF kernel matrix (B,N,N)  -- only its DIAGONAL is used,
             and dist_ii == 0 exactly, so cov_diag == 1 and the (B,N,N)
             work is mathematically dead. sigma_adjusted == sigma.
    return (mu, sigma_adjusted)

Strategy: data-parallel over batch B=8, one batch element per NeuronCore.
Per core: two linear heads over x_b [2048,1024] in bf16. The PE streams
one output column per cycle, so the floor is
   2 heads * (2048*512 outputs / 128 lanes) * (1024/128 k-tiles)
   = 131072 cycles ~= 54.6 us @ 2.4 GHz.

Schedule (v9) — measured DMA laws this is built around:
  * per-queue throughput ~95-105 B/ns when two queues run, ~200 solo;
    gpsimd (SWDGE) adds ~130 on top but starves the HWDGE queues, and
    its first transfer starts ~2.5us later than HWDGE.
  * descriptors are per partition-row; keep per-partition contiguous
    runs >=4KB or the descriptor generator (22ns/desc) caps the rate.
  * every DMA's completion semaphore lands ~2us after its last byte
    for 128-partition transfers — but ~4.5us for partition-SPLIT
    (64-row) transfers, so loads/stores are fine-grained by k-tile
    halves and column halves, never by partitions.
So:
 - OPENING: sync streams x_c0 as four kt-pairs (2KB descriptors) while
   scalar streams wlv as four kt-pairs; chunk 0 (kt-outer over 4 PSUM
   banks) consumes each pair as it lands, so the PE starts ~5.5us —
   right as the warmup ends — and chunk 0 co-completes with its own
   load (~15us, the 2MB/2-queue bandwidth floor).  x_c1 follows as
   kt-quads, wmu as kt-halves.  gpsimd carries only the biases, plus
   x_c2/x_c3 halves gated behind the last wlv pair (tiny copy) so the
   SWDGE burst can't starve the opening — they're needed only at
   ~21/~28us.
 - lv head: chunk-major (kt-outer, 4 PSUM banks); mu head: EB-MAJOR
   (kt-inner groups) so each eb's output completes 6.8us apart.
 - Stores are [128, 1024] column-halves (2 KB descriptors, full 128
   partitions): each (head, eb) stores chunks 01 as soon as c1's
   epilogue lands and chunks 23 at eb end, alternating queues.  After
   the last (tapered 256/128/128) matmul group only ONE 256 KB store
   remains -> post-compute tail ~5us + host-stop latency, instead of
   the 11us serialized store drains of chunk-pair stores.
 - Warmup matmuls (44) keep the PE busy through the DMA lead-in so the
   HAM clock gate reaches 2.4 GHz with no re-throttle gap.
 - Epilogues: one op per PSUM tile, bias via the per-partition port:
     sigma = Exp(PSUM * 0.5 + 0.5*b_lv[e])  on the Scalar engine
     mu    = PSUM + b_mu[e]                 on the Vector engine
   both writing bf16 into [128, 2048] out tiles ([p][eb][c][t] DRAM
   layout); host un-transposes and upcasts.
"""
import os
import sys

import numpy as np

try:
    import concourse.bass as bass  # noqa: F401
except Exception:  # pragma: no cover - path fallback for fresh dirs
    for p in ("/opt/trn_rl_repo", os.path.expanduser("~/.axon_site/_ro/trn_rl_repo")):
        if os.path.isdir(p) and p not in sys.path:
            sys.path.insert(0, p)
    import concourse.bass as bass

import ml_dtypes
import concourse.mybir as mybir
from concourse import bacc
from concourse.bass_utils import run_bass_kernel_spmd
from concourse.tile import TileContext

B, N, D, E = 8, 2048, 1024, 512
P = 128
KT = D // P          # 8 k-tiles
EB = E // P          # 4 embedding blocks
TC = N // 512        # 4 token chunks of 512
F32, BF16 = mybir.dt.float32, mybir.dt.bfloat16

_NC = None


def _build():
    nc = bacc.Bacc()
    # x packed on host as [p][c][kt][t] -> [P, KT*N]
    xP = nc.declare_dram_parameter("xP", [P, KT * N], BF16, isOutput=False)
    # weights packed as [p][kt][e] -> [P, KT*E]
    wlv = nc.declare_dram_parameter("wlv", [P, E * KT], BF16, isOutput=False)
    wmu = nc.declare_dram_parameter("wmu", [P, E * KT], BF16, isOutput=False)
    # biases arranged [P, EB]: element (p, eb) = bias[eb*128 + p]
    bmu = nc.declare_dram_parameter("bmu", [P, EB], F32, isOutput=False)
    blv = nc.declare_dram_parameter("blv", [P, EB], F32, isOutput=False)  # 0.5*b
    # outputs packed [p][eb][c][t]: element (p, eb*N + c*512 + t) =
    # head[c*512 + t, eb*128 + p]
    muT = nc.declare_dram_parameter("muT", [P, EB * N], BF16, isOutput=True)
    sgT = nc.declare_dram_parameter("sgT", [P, EB * N], BF16, isOutput=True)

    with TileContext(nc) as tc:
        with (
            tc.tile_pool(name="const", bufs=1) as cpool,
            tc.tile_pool(name="out", bufs=8) as opool,
            tc.tile_pool(name="psF", bufs=1, space="PSUM") as psF,
            tc.tile_pool(name="psM", bufs=7, space="PSUM") as psM,
        ):
            x_sb = [
                cpool.tile([P, KT, 512], BF16, name=f"x_sb{c}") for c in range(TC)
            ]
            wlv_sb = cpool.tile([P, KT, E], BF16)
            wmu_sb = cpool.tile([P, KT, E], BF16)
            blv_sb = cpool.tile([P, EB], F32)
            bmu_sb = cpool.tile([P, EB], F32)
            warm = cpool.tile([P, P], BF16)

            wlv_r = wlv[:, :].rearrange("p (kt e) -> p kt e", kt=KT)
            wmu_r = wmu[:, :].rearrange("p (kt e) -> p kt e", kt=KT)

            def xslab(c):
                off = c * 512 * KT
                return xP[:, off : off + 512 * KT].rearrange(
                    "p (kt t) -> p kt t", kt=KT
                )

            # Warmup: PE busy continuously from ~0.2us so the HAM clock
            # gate reaches 2.4 GHz before the real stream starts.
            nc.vector.memset(warm, 0)
            wps = psF.tile([P, 512], F32, tag="ps", name="warmps")
            for i in range(44):
                nc.tensor.matmul(
                    wps[:, 0:P], warm[:, :], warm[:, :],
                    start=(i == 0), stop=(i == 43),
                )

            # --- OPENING (v4-front) -----------------------------------
            # kt-half slabs (4KB descriptors) on the two HWDGE queues:
            # sync carries x_c0 h1/h2 then x_c1 h1 then wmu h1; scalar
            # carries wlv h1/h2 then x_c1 h2 then wmu h2.  First matmul
            # starts ~7us; chunk 0 (kt-outer) overlaps its own load.
            # gpsimd carries the biases now and x_c2/x_c3 gated behind
            # chunk-0's first output tile so SWDGE can't starve the
            # opening (they're needed only at ~21/~28).
            nc.sync.dma_start(out=x_sb[0][:, 0:2, :], in_=xslab(0)[:, 0:2, :])
            nc.scalar.dma_start(out=wlv_sb[:, 0:2, :], in_=wlv_r[:, 0:2, :])
            nc.sync.dma_start(out=x_sb[0][:, 2:4, :], in_=xslab(0)[:, 2:4, :])
            nc.scalar.dma_start(out=wlv_sb[:, 2:4, :], in_=wlv_r[:, 2:4, :])
            nc.sync.dma_start(out=x_sb[0][:, 4:6, :], in_=xslab(0)[:, 4:6, :])
            nc.scalar.dma_start(out=wlv_sb[:, 4:6, :], in_=wlv_r[:, 4:6, :])
            nc.sync.dma_start(out=x_sb[0][:, 6:KT, :], in_=xslab(0)[:, 6:KT, :])
            nc.scalar.dma_start(out=wlv_sb[:, 6:KT, :], in_=wlv_r[:, 6:KT, :])
            nc.sync.dma_start(out=x_sb[1][:, 0:2, :], in_=xslab(1)[:, 0:2, :])
            nc.scalar.dma_start(out=x_sb[1][:, 4:6, :], in_=xslab(1)[:, 4:6, :])
            nc.sync.dma_start(out=x_sb[1][:, 2:4, :], in_=xslab(1)[:, 2:4, :])
            nc.scalar.dma_start(out=x_sb[1][:, 6:KT, :], in_=xslab(1)[:, 6:KT, :])
            nc.sync.dma_start(out=wmu_sb[:, 0:4, :], in_=wmu_r[:, 0:4, :])
            nc.scalar.dma_start(out=wmu_sb[:, 4:KT, :], in_=wmu_r[:, 4:KT, :])
            nc.gpsimd.dma_start(out=blv_sb, in_=blv[:, :])
            nc.gpsimd.dma_start(out=bmu_sb, in_=bmu[:, :])

            EXP = mybir.ActivationFunctionType.Exp

            def epilogue(hname, bias_sb, eb, ps, ot, o0, ow):
                """PSUM -> bf16 slice [o0:o0+ow) of the [P, N] out tile."""
                osl = ot[:, o0 : o0 + ow]
                if hname == "lv":
                    nc.scalar.activation(
                        osl, ps, EXP, bias=bias_sb[:, eb : eb + 1], scale=0.5
                    )
                else:
                    nc.vector.tensor_scalar_add(osl, ps, bias_sb[:, eb : eb + 1])

            def store_half(outdram, eb, half, ot, q):
                """Store [P, 1024] column-half (chunks 2h,2h+1) of an eb."""
                ds = slice(eb * N + half * 1024, eb * N + (half + 1) * 1024)
                if q == 0:
                    nc.sync.dma_start(out=outdram[:, ds], in_=ot[:, half * 1024 : (half + 1) * 1024])
                else:
                    nc.scalar.dma_start(out=outdram[:, ds], in_=ot[:, half * 1024 : (half + 1) * 1024])

            # --- lv head: chunk-major, kt-outer -----------------------
            ot_lv = [
                opool.tile([P, N], BF16, tag="o", name=f"o_lv{eb}")
                for eb in range(EB)
            ]
            for c in range(TC):
                pss = [
                    psM.tile([P, 512], F32, tag="ps", name=f"ps_lv{c}{eb}")
                    for eb in range(EB)
                ]
                for kt in range(KT):
                    for eb in range(EB):
                        nc.tensor.matmul(
                            pss[eb],
                            wlv_sb[:, kt, eb * P : (eb + 1) * P],
                            x_sb[c][:, kt, :],
                            start=(kt == 0),
                            stop=(kt == KT - 1),
                        )
                for eb in range(EB):
                    epilogue("lv", blv_sb, eb, pss[eb], ot_lv[eb], c * 512, 512)
                if c == 0:
                    # Gate the gpsimd x_c2/x_c3 loads behind the last
                    # wlv slab so the SWDGE burst can't starve the
                    # opening slabs (needed only at ~21/~28us); halves
                    # so c2/c3's first kt-steps don't wait whole chunks.
                    nc.gpsimd.tensor_copy(warm[:, 0:2], wlv_sb[:, KT - 1, 0:2])
                    nc.gpsimd.dma_start(out=x_sb[2][:, 0:4, :], in_=xslab(2)[:, 0:4, :])
                    nc.gpsimd.dma_start(out=x_sb[2][:, 4:KT, :], in_=xslab(2)[:, 4:KT, :])
                    nc.gpsimd.dma_start(out=x_sb[3][:, 0:4, :], in_=xslab(3)[:, 0:4, :])
                    nc.gpsimd.dma_start(out=x_sb[3][:, 4:KT, :], in_=xslab(3)[:, 4:KT, :])
                if c == 1 or c == 3:
                    for eb in range(EB):
                        store_half(sgT, eb, c // 2, ot_lv[eb], eb % 2)

            # --- mu head: EB-MAJOR, kt-inner; store halves as each
            # half-eb finishes so only eb3's last half trails the end --
            for eb in range(EB):
                ot = opool.tile([P, N], BF16, tag="o", name=f"o_mu{eb}")
                for c in range(TC):
                    pieces = (
                        [(0, 512)]
                        if not (eb == EB - 1 and c == TC - 1)
                        else [(0, 256), (256, 128), (384, 128)]
                    )
                    for o0, ow in pieces:
                        ps = psM.tile(
                            [P, ow], F32, tag="ps", name=f"ps_mu{eb}{c}_{o0}"
                        )
                        for kt in range(KT):
                            nc.tensor.matmul(
                                ps,
                                wmu_sb[:, kt, eb * P : (eb + 1) * P],
                                x_sb[c][:, kt, o0 : o0 + ow],
                                start=(kt == 0),
                                stop=(kt == KT - 1),
                            )
                        epilogue("mu", bmu_sb, eb, ps, ot, c * 512 + o0, ow)
                    if c == 1 or c == 3:
                        store_half(muT, eb, c // 2, ot, (eb + c // 2) % 2)
    nc.compile()
    return nc


def _pack_x(xb):
    """xb [N, D] f32 -> [P, KT*N] bf16 packed as [p][c][kt][t]."""
    xt = xb.T.astype(ml_dtypes.bfloat16).reshape(KT, P, TC, 512)  # [kt, p, c, t]
    return np.ascontiguousarray(xt.transpose(1, 2, 0, 3).reshape(P, KT * N))


def _pack_w(W):
    """W [E, D] f32 -> [P, KT*E] bf16 packed as [p][kt][e]."""
    wt = W.astype(np.float32).T.astype(ml_dtypes.bfloat16)   # [D, E]
    v = wt.reshape(KT, P, E)
    return np.ascontiguousarray(v.transpose(1, 0, 2).reshape(P, KT * E))


def _unpack_out(a):
    """[P, EB*N] bf16 packed [p][eb][c][t] -> [N, E] f32."""
    v = a.reshape(P, EB, N)                      # [p, eb, n]
    return np.ascontiguousarray(v.transpose(2, 1, 0).reshape(N, E)).astype(np.float32)


def run(x, W_mu, b_mu, W_logvar, b_logvar, trace=False, **trace_kwargs):
    global _NC
    if _NC is None:
        _NC = _build()

    x = np.asarray(x, dtype=np.float32)
    wlv_h = _pack_w(np.asarray(W_logvar))
    wmu_h = _pack_w(np.asarray(W_mu))
    bmu_h = np.ascontiguousarray(np.asarray(b_mu, dtype=np.float32).reshape(EB, P).T)
    blv_h = np.ascontiguousarray(
        (0.5 * np.asarray(b_logvar, dtype=np.float32)).reshape(EB, P).T
    )

    in_maps = [
        {
            "xP": _pack_x(x[b]),
            "wlv": wlv_h,
            "wmu": wmu_h,
            "bmu": bmu_h,
            "blv": blv_h,
        }
        for b in range(B)
    ]
    res = run_bass_kernel_spmd(
        _NC, in_maps, core_ids=list(range(B)), trace=trace, **trace_kwargs
    )
    mu = np.stack([_unpack_out(res.results[b]["muT"]) for b in range(B)])
    sigma = np.stack([_unpack_out(res.results[b]["sgT"]) for b in range(B)])
    return (mu, sigma), res


def kernel(x, W_mu, b_mu, W_logvar, b_logvar):
    (mu, sigma), _ = run(x, W_mu, b_mu, W_logvar, b_logvar, trace=False)
    return mu, sigma


# revision 23
# speedup vs baseline: 1.0011x; 1.0011x over previous
"""Trainium2 Bass kernel for GaussianProcessEmbeddingHead.

The reference computes:
    mu     = x @ W_mu.T + b_mu                      (B,N,E)
    sigma  = exp(0.5*(x @ W_logvar.T + b_logvar))   (B,N,E)
    K      = RBF kernel matrix (B,N,N)  -- only its DIAGONAL is used,
             and dist_ii == 0 exactly, so cov_diag == 1 and the (B,N,N)
             work is mathematically dead. sigma_adjusted == sigma.
    return (mu, sigma_adjusted)

Strategy: data-parallel over batch B=8, one batch element per NeuronCore.
Per core: two linear heads over x_b [2048,1024] in bf16. The PE streams
one output column per cycle, so the floor is
   2 heads * (2048*512 outputs / 128 lanes) * (1024/128 k-tiles)
   = 131072 cycles ~= 54.6 us @ 2.4 GHz.

Schedule (v9) — measured DMA laws this is built around:
  * per-queue throughput ~95-105 B/ns when two queues run, ~200 solo;
    gpsimd (SWDGE) adds ~130 on top but starves the HWDGE queues, and
    its first transfer starts ~2.5us later than HWDGE.
  * descriptors are per partition-row; keep per-partition contiguous
    runs >=4KB or the descriptor generator (22ns/desc) caps the rate.
  * every DMA's completion semaphore lands ~2us after its last byte
    for 128-partition transfers — but ~4.5us for partition-SPLIT
    (64-row) transfers, so loads/stores are fine-grained by k-tile
    halves and column halves, never by partitions.
So:
 - OPENING: sync streams x_c0 as four kt-pairs (2KB descriptors) while
   scalar streams wlv as four kt-pairs; chunk 0 (kt-outer over 4 PSUM
   banks) consumes each pair as it lands, so the PE starts ~5.5us —
   right as the warmup ends — and chunk 0 co-completes with its own
   load (~15us, the 2MB/2-queue bandwidth floor).  x_c1 follows as
   kt-quads, wmu as kt-halves.  gpsimd carries only the biases, plus
   x_c2/x_c3 halves gated behind the last wlv pair (tiny copy) so the
   SWDGE burst can't starve the opening — they're needed only at
   ~21/~28us.
 - lv head: chunk-major (kt-outer, 4 PSUM banks); mu head: EB-MAJOR
   (kt-inner groups) so each eb's output completes 6.8us apart.
 - Stores are [128, 1024] column-halves (2 KB descriptors, full 128
   partitions): each (head, eb) stores chunks 01 as soon as c1's
   epilogue lands and chunks 23 at eb end, alternating queues.  After
   the last (tapered 256/128/128) matmul group only ONE 256 KB store
   remains -> post-compute tail ~5us + host-stop latency, instead of
   the 11us serialized store drains of chunk-pair stores.
 - Warmup matmuls (44) keep the PE busy through the DMA lead-in so the
   HAM clock gate reaches 2.4 GHz with no re-throttle gap.
 - Epilogues: one op per PSUM tile, bias via the per-partition port:
     sigma = Exp(PSUM * 0.5 + 0.5*b_lv[e])  on the Scalar engine
     mu    = PSUM + b_mu[e]                 on the Vector engine
   both writing bf16 into [128, 2048] out tiles ([p][eb][c][t] DRAM
   layout); host un-transposes and upcasts.
"""
import os
import sys

import numpy as np

try:
    import concourse.bass as bass  # noqa: F401
except Exception:  # pragma: no cover - path fallback for fresh dirs
    for p in ("/opt/trn_rl_repo", os.path.expanduser("~/.axon_site/_ro/trn_rl_repo")):
        if os.path.isdir(p) and p not in sys.path:
            sys.path.insert(0, p)
    import concourse.bass as bass

import ml_dtypes
import concourse.mybir as mybir
from concourse import bacc
from concourse.bass_utils import run_bass_kernel_spmd
from concourse.tile import TileContext

B, N, D, E = 8, 2048, 1024, 512
P = 128
KT = D // P          # 8 k-tiles
EB = E // P          # 4 embedding blocks
TC = N // 512        # 4 token chunks of 512
F32, BF16 = mybir.dt.float32, mybir.dt.bfloat16

_NC = None


def _build():
    nc = bacc.Bacc()
    # x packed on host as [p][c][kt][t] -> [P, KT*N]
    xP = nc.declare_dram_parameter("xP", [P, KT * N], BF16, isOutput=False)
    # weights packed as [p][kt][e] -> [P, KT*E]
    wlv = nc.declare_dram_parameter("wlv", [P, E * KT], BF16, isOutput=False)
    wmu = nc.declare_dram_parameter("wmu", [P, E * KT], BF16, isOutput=False)
    # biases arranged [P, EB]: element (p, eb) = bias[eb*128 + p]
    bmu = nc.declare_dram_parameter("bmu", [P, EB], F32, isOutput=False)
    blv = nc.declare_dram_parameter("blv", [P, EB], F32, isOutput=False)  # 0.5*b
    # outputs packed [p][eb][c][t]: element (p, eb*N + c*512 + t) =
    # head[c*512 + t, eb*128 + p]
    muT = nc.declare_dram_parameter("muT", [P, EB * N], BF16, isOutput=True)
    sgT = nc.declare_dram_parameter("sgT", [P, EB * N], BF16, isOutput=True)

    with TileContext(nc) as tc:
        with (
            tc.tile_pool(name="const", bufs=1) as cpool,
            tc.tile_pool(name="out", bufs=8) as opool,
            tc.tile_pool(name="psF", bufs=1, space="PSUM") as psF,
            tc.tile_pool(name="psM", bufs=7, space="PSUM") as psM,
        ):
            x_sb = [
                cpool.tile([P, KT, 512], BF16, name=f"x_sb{c}") for c in range(TC)
            ]
            wlv_sb = cpool.tile([P, KT, E], BF16)
            wmu_sb = cpool.tile([P, KT, E], BF16)
            blv_sb = cpool.tile([P, EB], F32)
            bmu_sb = cpool.tile([P, EB], F32)
            warm = cpool.tile([P, P], BF16)

            wlv_r = wlv[:, :].rearrange("p (kt e) -> p kt e", kt=KT)
            wmu_r = wmu[:, :].rearrange("p (kt e) -> p kt e", kt=KT)

            def xslab(c):
                off = c * 512 * KT
                return xP[:, off : off + 512 * KT].rearrange(
                    "p (kt t) -> p kt t", kt=KT
                )

            # Warmup: PE busy continuously from ~0.2us so the HAM clock
            # gate reaches 2.4 GHz before the real stream starts.
            nc.vector.memset(warm, 0)
            wps = psF.tile([P, 512], F32, tag="ps", name="warmps")
            for i in range(44):
                nc.tensor.matmul(
                    wps[:, 0:P], warm[:, :], warm[:, :],
                    start=(i == 0), stop=(i == 43),
                )

            # --- OPENING (v4-front) -----------------------------------
            # kt-half slabs (4KB descriptors) on the two HWDGE queues:
            # sync carries x_c0 h1/h2 then x_c1 h1 then wmu h1; scalar
            # carries wlv h1/h2 then x_c1 h2 then wmu h2.  First matmul
            # starts ~7us; chunk 0 (kt-outer) overlaps its own load.
            # gpsimd carries the biases now and x_c2/x_c3 gated behind
            # chunk-0's first output tile so SWDGE can't starve the
            # opening (they're needed only at ~21/~28).
            nc.sync.dma_start(out=x_sb[0][:, 0:2, :], in_=xslab(0)[:, 0:2, :])
            nc.scalar.dma_start(out=wlv_sb[:, 0:2, :], in_=wlv_r[:, 0:2, :])
            nc.sync.dma_start(out=x_sb[0][:, 2:4, :], in_=xslab(0)[:, 2:4, :])
            nc.scalar.dma_start(out=wlv_sb[:, 2:4, :], in_=wlv_r[:, 2:4, :])
            nc.sync.dma_start(out=x_sb[0][:, 4:6, :], in_=xslab(0)[:, 4:6, :])
            nc.scalar.dma_start(out=wlv_sb[:, 4:6, :], in_=wlv_r[:, 4:6, :])
            # the LAST-needed chunk-0 pieces ride gpsimd — it is otherwise
            # idle 3-13us, so this adds ~130 B/ns to the opening window
            # without gating the early kt-steps.
            nc.gpsimd.dma_start(out=x_sb[0][:, 6:KT, :], in_=xslab(0)[:, 6:KT, :])
            nc.gpsimd.dma_start(out=wlv_sb[:, 6:KT, :], in_=wlv_r[:, 6:KT, :])
            nc.sync.dma_start(out=x_sb[1][:, 0:2, :], in_=xslab(1)[:, 0:2, :])
            nc.scalar.dma_start(out=x_sb[1][:, 4:6, :], in_=xslab(1)[:, 4:6, :])
            nc.sync.dma_start(out=x_sb[1][:, 2:4, :], in_=xslab(1)[:, 2:4, :])
            nc.scalar.dma_start(out=x_sb[1][:, 6:KT, :], in_=xslab(1)[:, 6:KT, :])
            nc.sync.dma_start(out=wmu_sb[:, 0:4, :], in_=wmu_r[:, 0:4, :])
            nc.scalar.dma_start(out=wmu_sb[:, 4:KT, :], in_=wmu_r[:, 4:KT, :])
            nc.gpsimd.dma_start(out=blv_sb, in_=blv[:, :])
            nc.gpsimd.dma_start(out=bmu_sb, in_=bmu[:, :])

            EXP = mybir.ActivationFunctionType.Exp

            def epilogue(hname, bias_sb, eb, ps, ot, o0, ow):
                """PSUM -> bf16 slice [o0:o0+ow) of the [P, N] out tile."""
                osl = ot[:, o0 : o0 + ow]
                if hname == "lv":
                    nc.scalar.activation(
                        osl, ps, EXP, bias=bias_sb[:, eb : eb + 1], scale=0.5
                    )
                else:
                    nc.vector.tensor_scalar_add(osl, ps, bias_sb[:, eb : eb + 1])

            def store_half(outdram, eb, half, ot, q):
                """Store [P, 1024] column-half (chunks 2h,2h+1) of an eb."""
                ds = slice(eb * N + half * 1024, eb * N + (half + 1) * 1024)
                if q == 0:
                    nc.sync.dma_start(out=outdram[:, ds], in_=ot[:, half * 1024 : (half + 1) * 1024])
                else:
                    nc.scalar.dma_start(out=outdram[:, ds], in_=ot[:, half * 1024 : (half + 1) * 1024])

            # --- lv head: chunk-major, kt-outer -----------------------
            ot_lv = [
                opool.tile([P, N], BF16, tag="o", name=f"o_lv{eb}")
                for eb in range(EB)
            ]
            for c in range(TC):
                pss = [
                    psM.tile([P, 512], F32, tag="ps", name=f"ps_lv{c}{eb}")
                    for eb in range(EB)
                ]
                for kt in range(KT):
                    for eb in range(EB):
                        nc.tensor.matmul(
                            pss[eb],
                            wlv_sb[:, kt, eb * P : (eb + 1) * P],
                            x_sb[c][:, kt, :],
                            start=(kt == 0),
                            stop=(kt == KT - 1),
                        )
                for eb in range(EB):
                    epilogue("lv", blv_sb, eb, pss[eb], ot_lv[eb], c * 512, 512)
                if c == 0:
                    # Gate the gpsimd x_c2/x_c3 loads behind the last
                    # wlv slab so the SWDGE burst can't starve the
                    # opening slabs (needed only at ~21/~28us); halves
                    # so c2/c3's first kt-steps don't wait whole chunks.
                    nc.gpsimd.tensor_copy(warm[:, 0:2], wlv_sb[:, KT - 1, 0:2])
                    nc.gpsimd.dma_start(out=x_sb[2][:, 0:4, :], in_=xslab(2)[:, 0:4, :])
                    nc.gpsimd.dma_start(out=x_sb[2][:, 4:KT, :], in_=xslab(2)[:, 4:KT, :])
                    nc.gpsimd.dma_start(out=x_sb[3][:, 0:4, :], in_=xslab(3)[:, 0:4, :])
                    nc.gpsimd.dma_start(out=x_sb[3][:, 4:KT, :], in_=xslab(3)[:, 4:KT, :])
                if c == 1 or c == 3:
                    for eb in range(EB):
                        store_half(sgT, eb, c // 2, ot_lv[eb], eb % 2)

            # --- mu head: EB-MAJOR, kt-inner; store halves as each
            # half-eb finishes so only eb3's last half trails the end --
            for eb in range(EB):
                ot = opool.tile([P, N], BF16, tag="o", name=f"o_mu{eb}")
                for c in range(TC):
                    pieces = (
                        [(0, 512)]
                        if not (eb == EB - 1 and c == TC - 1)
                        else [(0, 256), (256, 128), (384, 128)]
                    )
                    for o0, ow in pieces:
                        ps = psM.tile(
                            [P, ow], F32, tag="ps", name=f"ps_mu{eb}{c}_{o0}"
                        )
                        for kt in range(KT):
                            nc.tensor.matmul(
                                ps,
                                wmu_sb[:, kt, eb * P : (eb + 1) * P],
                                x_sb[c][:, kt, o0 : o0 + ow],
                                start=(kt == 0),
                                stop=(kt == KT - 1),
                            )
                        epilogue("mu", bmu_sb, eb, ps, ot, c * 512 + o0, ow)
                    if c == 1 or c == 3:
                        store_half(muT, eb, c // 2, ot, (eb + c // 2) % 2)
    nc.compile()
    return nc


def _pack_x(xb):
    """xb [N, D] f32 -> [P, KT*N] bf16 packed as [p][c][kt][t]."""
    xt = xb.T.astype(ml_dtypes.bfloat16).reshape(KT, P, TC, 512)  # [kt, p, c, t]
    return np.ascontiguousarray(xt.transpose(1, 2, 0, 3).reshape(P, KT * N))


def _pack_w(W):
    """W [E, D] f32 -> [P, KT*E] bf16 packed as [p][kt][e]."""
    wt = W.astype(np.float32).T.astype(ml_dtypes.bfloat16)   # [D, E]
    v = wt.reshape(KT, P, E)
    return np.ascontiguousarray(v.transpose(1, 0, 2).reshape(P, KT * E))


def _unpack_out(a):
    """[P, EB*N] bf16 packed [p][eb][c][t] -> [N, E] f32."""
    v = a.reshape(P, EB, N)                      # [p, eb, n]
    return np.ascontiguousarray(v.transpose(2, 1, 0).reshape(N, E)).astype(np.float32)


def run(x, W_mu, b_mu, W_logvar, b_logvar, trace=False, **trace_kwargs):
    global _NC
    if _NC is None:
        _NC = _build()

    x = np.asarray(x, dtype=np.float32)
    wlv_h = _pack_w(np.asarray(W_logvar))
    wmu_h = _pack_w(np.asarray(W_mu))
    bmu_h = np.ascontiguousarray(np.asarray(b_mu, dtype=np.float32).reshape(EB, P).T)
    blv_h = np.ascontiguousarray(
        (0.5 * np.asarray(b_logvar, dtype=np.float32)).reshape(EB, P).T
    )

    in_maps = [
        {
            "xP": _pack_x(x[b]),
            "wlv": wlv_h,
            "wmu": wmu_h,
            "bmu": bmu_h,
            "blv": blv_h,
        }
        for b in range(B)
    ]
    res = run_bass_kernel_spmd(
        _NC, in_maps, core_ids=list(range(B)), trace=trace, **trace_kwargs
    )
    mu = np.stack([_unpack_out(res.results[b]["muT"]) for b in range(B)])
    sigma = np.stack([_unpack_out(res.results[b]["sgT"]) for b in range(B)])
    return (mu, sigma), res


def kernel(x, W_mu, b_mu, W_logvar, b_logvar):
    (mu, sigma), _ = run(x, W_mu, b_mu, W_logvar, b_logvar, trace=False)
    return mu, sigma
